# revision 7
# baseline (speedup 1.0000x reference)
"""GAT layer (nn_GATLayer) on 8 Trainium2 NeuronCores.

Math (per batch b, with h = x@W, s1 = h@a1, s2 = h@a2):
    e[i,j] = lrelu_0.2(s1_i + s2_j), masked by adj[i,j], softmax over j
    out    = attn @ h

Key identity: softmax over j is invariant to any per-i scale, and
exp(lrelu(y)) = max(exp(y), exp(0.2 y)). Dividing column i by e^{0.2 s1_i}:
    P'[j,i] = max(e^{0.8 s1_i} * e^{s2_j},  e^{0.2 s2_j}) * adj[i,j]
-- the i-dependence is a rank-1 product: no N^2 transcendentals at all.

Device formulation (per core = one batch element), [p=j, f=i] layout:
    q   = (E1b mult E2_j) max F2_j     (tensor_scalar, 4x DVE perf mode)
    P'' = mask applied to q:
      GAT_ACCUM=1: gpsimd DMA of the {0,BIG} mask with accum_op=min,
                   RMW directly onto q -- no vector op at all
      GAT_ACCUM=0: tensor_tensor mult with a {0,1} bf16 mask (2x DVE),
                   a few tiles offloaded to gpsimd
    numT[d,i] = sum_j hcat[j,d] P''[j,i],  hcat = [h | 1]  (PE, bf16)
Host computes h/s1/s2/exp vectors (O(N D^2 + N) work) and the final
divide+transpose out[i,d] = numT[d,i]/numT[64,i].

Sharding: data-parallel over batch B=8 across the 8 cores; mask (shared)
replicated. All N^2 element work uses standard DVE ops that hit the
2x/4x perf modes (bf16, packed, SBUF) -- custom DVE ops can't.
"""

import os
import sys

sys.path.insert(0, "/opt/trn_rl_repo")

import numpy as np
import ml_dtypes

B, N, DIN, DOUT = 8, 2048, 64, 64
NCORES = 8
PJ = 128              # j-tile partition size
NJT = N // PJ         # 16 j-tiles
FCH = 512             # psum bank chunk (fp32)
NCH = N // FCH        # 4 chunks of the free dim
HC = DOUT + 2         # hcat stride: 64 h cols + 1 ones col + 1 pad
MASK_BIG = 1.0e6      # unmasked value for the min-accum mask path
GP_TILES = (2, 5, 8, 11, 14)   # mask-mult tiles offloaded to gpsimd

ACCUM = bool(int(os.environ.get("GAT_ACCUM", "0")))

_COMPILED = None
LAST_RESULT = None    # BassKernelResults from the last run (for test.py)


def _build_nc():
    """Build the Bass module (shared SPMD program for all 8 cores)."""
    from contextlib import ExitStack

    import concourse.tile as tile
    from concourse import bacc, mybir

    f32 = mybir.dt.float32
    bf16 = mybir.dt.bfloat16
    ALU = mybir.AluOpType

    nc = bacc.Bacc("TRN2", target_bir_lowering=False, debug=False, num_devices=NCORES)

    maskt = nc.dram_tensor("maskt", [N, N], bf16, kind="ExternalInput").ap()
    hcat = nc.dram_tensor("hcat", [PJ, NJT * HC], bf16, kind="ExternalInput").ap()
    e1b = nc.dram_tensor("e1b", [PJ, N], bf16, kind="ExternalInput").ap()
    e2f2 = nc.dram_tensor("e2f2", [PJ, 2 * NJT], f32, kind="ExternalInput").ap()
    out = nc.dram_tensor("out", [DOUT + 1, N], f32, kind="ExternalOutput").ap()

    with ExitStack() as ctx:
        tc = ctx.enter_context(tile.TileContext(nc))

        const = ctx.enter_context(tc.tile_pool(name="const", bufs=1))
        big = ctx.enter_context(tc.tile_pool(name="big", bufs=1))

        # ---- small inputs first on the queue, then the mask stream ----
        e2f2_sb = const.tile([PJ, 2 * NJT], f32, tag="e2f2")
        nc.sync.dma_start(e2f2_sb[:], e2f2)
        e1b_sb = big.tile([PJ, N], bf16, tag="e1b")
        nc.sync.dma_start(e1b_sb[:], e1b)
        hcat_sb = const.tile([PJ, NJT * HC], bf16, tag="hcat")
        nc.sync.dma_start(hcat_sb[:], hcat)

        num_pool = ctx.enter_context(
            tc.tile_pool(name="num_psum", bufs=1, space="PSUM")
        )
        numT_ps = num_pool.tile([DOUT + 1, N], f32, tag="numt")

        def emit_matmuls(t, src, first, last):
            lhsT = hcat_sb[:, t * HC : t * HC + DOUT + 1]
            for c in range(NCH):
                sl = slice(c * FCH, (c + 1) * FCH)
                nc.tensor.matmul(
                    numT_ps[:, sl], lhsT, src[:, sl],
                    start=first, stop=last,
                )

        if ACCUM:
            # mask folded into a gpsimd (SWDGE) DMA: q = min(q, mask{0,BIG})
            qpool = ctx.enter_context(tc.tile_pool(name="q", bufs=6))
            for t in range(NJT):
                q_sb = qpool.tile([PJ, N], bf16, tag="q")
                nc.vector.tensor_scalar(
                    q_sb[:],
                    e1b_sb[:],
                    e2f2_sb[:, t : t + 1],
                    e2f2_sb[:, NJT + t : NJT + t + 1],
                    op0=ALU.mult,
                    op1=ALU.max,
                )
                acc = nc.gpsimd.dma_start(
                    q_sb[:],
                    maskt[t * PJ : (t + 1) * PJ, :],
                    accum_op=ALU.mult,
                )
                acc.ins.mode = "CCE"  # walrus rejects cce_op under Copy mode
                emit_matmuls(t, q_sb, first=(t == 0), last=(t == NJT - 1))
        else:
            mpool = ctx.enter_context(tc.tile_pool(name="mask", bufs=NJT))
            mask_sb = []
            for t in range(NJT):
                mb_t = mpool.tile([PJ, N], bf16, tag="mb")
                nc.sync.dma_start(mb_t[:], maskt[t * PJ : (t + 1) * PJ, :])
                mask_sb.append(mb_t)

            qpool = ctx.enter_context(tc.tile_pool(name="q", bufs=3))
            ppool = ctx.enter_context(tc.tile_pool(name="probs", bufs=3))
            gpool = ctx.enter_context(
                tc.tile_pool(name="gp_probs", bufs=len(GP_TILES))
            )
            gp_srcs = {}
            dve_tiles = [t for t in range(NJT) if t not in GP_TILES]
            for t in range(NJT):
                q_sb = qpool.tile([PJ, N], bf16, tag="q")
                nc.vector.tensor_scalar(
                    q_sb[:],
                    e1b_sb[:],
                    e2f2_sb[:, t : t + 1],
                    e2f2_sb[:, NJT + t : NJT + t + 1],
                    op0=ALU.mult,
                    op1=ALU.max,
                )
                if t in GP_TILES:
                    p_sb = gpool.tile([PJ, N], bf16, tag="gp_p")
                    nc.gpsimd.tensor_tensor(
                        p_sb[:], mask_sb[t][:], q_sb[:], op=ALU.mult
                    )
                    gp_srcs[t] = p_sb
                else:
                    p_sb = ppool.tile([PJ, N], bf16, tag="p")
                    nc.vector.tensor_tensor(
                        p_sb[:], mask_sb[t][:], q_sb[:], op=ALU.mult
                    )
                    emit_matmuls(t, p_sb, first=(t == 0), last=False)
            # gpsimd tiles' matmuls go last so the in-order PE queue never
            # stalls on the slower gpsimd multiplies mid-stream
            for k, t in enumerate(GP_TILES):
                emit_matmuls(
                    t, gp_srcs[t], first=False, last=(k == len(GP_TILES) - 1)
                )

        # ---- drain numT and store; divide+transpose happen on host ----
        numt_sb = big.tile([DOUT + 1, N], f32, tag="numt_sb")
        for c in range(NCH):
            sl = slice(c * FCH, (c + 1) * FCH)
            nc.scalar.copy(numt_sb[:, sl], numT_ps[:, sl])
            nc.sync.dma_start(out[:, sl], numt_sb[:, sl])

    nc.compile()
    return nc


def _prep_inputs(x, adj, W, a):
    bf = ml_dtypes.bfloat16
    x = np.asarray(x, dtype=np.float32)
    W = np.ascontiguousarray(np.asarray(W, dtype=np.float32))
    a = np.asarray(a, dtype=np.float32)

    h = x @ W                                   # [B,N,DOUT]
    s1 = h @ a[:DOUT]                           # [B,N]
    s2 = h @ a[DOUT:]                           # [B,N]

    adjT = np.asarray(adj).T > 0                # [j,i] layout
    mask_bf = adjT.astype(bf)                   # {0,1}

    in_maps = []
    for b in range(NCORES):
        hcat = np.zeros((N, HC), dtype=bf)
        hcat[:, :DOUT] = h[b].astype(bf)
        hcat[:, DOUT] = bf(1.0)
        # pre-tiled [128, 16*66]: partition p, tile t = row t*128+p
        hcat_t = np.ascontiguousarray(
            hcat.reshape(NJT, PJ, HC).transpose(1, 0, 2).reshape(PJ, NJT * HC)
        )
        e1p = np.exp(0.8 * s1[b]).astype(bf)
        e1b = np.ascontiguousarray(np.broadcast_to(e1p[None, :], (PJ, N)))
        e2 = np.exp(s2[b]).astype(np.float32).reshape(NJT, PJ).T
        f2 = np.exp(0.2 * s2[b]).astype(np.float32).reshape(NJT, PJ).T
        e2f2 = np.ascontiguousarray(np.concatenate([e2, f2], axis=1))
        in_maps.append(
            {"maskt": mask_bf, "hcat": hcat_t, "e1b": e1b, "e2f2": e2f2}
        )
    return in_maps


def kernel(x, adj, W, a):
    global _COMPILED, LAST_RESULT
    from concourse import bass_utils

    x = np.asarray(x)
    adj = np.asarray(adj)
    assert x.shape == (B, N, DIN) and adj.shape == (N, N)

    if _COMPILED is None:
        _COMPILED = _build_nc()
    nc = _COMPILED

    in_maps = _prep_inputs(x, adj, W, a)
    res = bass_utils.run_bass_kernel_spmd(
        nc,
        in_maps,
        core_ids=list(range(NCORES)),
        trace=bool(int(os.environ.get("GAT_TRACE", "0"))),
    )
    LAST_RESULT = res
    out = np.empty((B, N, DOUT), dtype=np.float32)
    for b in range(NCORES):
        numt = res.results[b]["out"]            # [DOUT+1, N] f32
        out[b] = (numt[:DOUT] / numt[DOUT : DOUT + 1]).T
    return out


# revision 9
# speedup vs baseline: 1.0958x; 1.0958x over previous
"""GAT layer (nn_GATLayer) on 8 Trainium2 NeuronCores.

Math (per batch b, with h = x@W, s1 = h@a1, s2 = h@a2):
    e[i,j] = lrelu_0.2(s1_i + s2_j), masked by adj[i,j], softmax over j
    out    = attn @ h

Key identity: softmax over j is invariant to any per-i scale, and
exp(lrelu(y)) = max(exp(y), exp(0.2 y)). Dividing column i by e^{0.2 s1_i}:
    P'[j,i] = max(e^{0.8 s1_i} * e^{s2_j},  e^{0.2 s2_j}) * adj[i,j]
-- the i-dependence is a rank-1 product: no N^2 transcendentals at all.

Device formulation (per core = one batch element), [p=j, f=i] layout:
    q   = (E1b mult E2_j) max F2_j     (tensor_scalar, 4x DVE perf mode)
    P'' = mask applied to q:
      GAT_ACCUM=1: gpsimd DMA of the {0,BIG} mask with accum_op=min,
                   RMW directly onto q -- no vector op at all
      GAT_ACCUM=0: tensor_tensor mult with a {0,1} bf16 mask (2x DVE),
                   a few tiles offloaded to gpsimd
    numT[d,i] = sum_j hcat[j,d] P''[j,i],  hcat = [h | 1]  (PE, bf16)
Host computes h/s1/s2/exp vectors (O(N D^2 + N) work) and the final
divide+transpose out[i,d] = numT[d,i]/numT[64,i].

Sharding: data-parallel over batch B=8 across the 8 cores; mask (shared)
replicated. All N^2 element work uses standard DVE ops that hit the
2x/4x perf modes (bf16, packed, SBUF) -- custom DVE ops can't.
"""

import os
import sys

sys.path.insert(0, "/opt/trn_rl_repo")

import numpy as np
import ml_dtypes

B, N, DIN, DOUT = 8, 2048, 64, 64
NCORES = 8
PJ = 128              # j-tile partition size
NJT = N // PJ         # 16 j-tiles
FCH = 512             # psum bank chunk (fp32)
NCH = N // FCH        # 4 chunks of the free dim
HC = DOUT + 2         # hcat stride: 64 h cols + 1 ones col + 1 pad
MASK_BIG = 1.0e6      # unmasked value for the min-accum mask path
GP_TILES = (2, 6, 10, 14)      # mask-mult tiles offloaded to gpsimd

ACCUM = bool(int(os.environ.get("GAT_ACCUM", "0")))

_COMPILED = None
LAST_RESULT = None    # BassKernelResults from the last run (for test.py)


def _build_nc():
    """Build the Bass module (shared SPMD program for all 8 cores)."""
    from contextlib import ExitStack

    import concourse.tile as tile
    from concourse import bacc, mybir

    f32 = mybir.dt.float32
    bf16 = mybir.dt.bfloat16
    ALU = mybir.AluOpType

    nc = bacc.Bacc("TRN2", target_bir_lowering=False, debug=False, num_devices=NCORES)

    maskt = nc.dram_tensor("maskt", [N, N], bf16, kind="ExternalInput").ap()
    hcat = nc.dram_tensor("hcat", [PJ, NJT * HC], bf16, kind="ExternalInput").ap()
    e1b = nc.dram_tensor("e1b", [PJ, N], bf16, kind="ExternalInput").ap()
    e2f2 = nc.dram_tensor("e2f2", [PJ, 2 * NJT], f32, kind="ExternalInput").ap()
    out = nc.dram_tensor("out", [DOUT + 1, N], f32, kind="ExternalOutput").ap()

    with ExitStack() as ctx:
        tc = ctx.enter_context(tile.TileContext(nc))

        const = ctx.enter_context(tc.tile_pool(name="const", bufs=1))
        big = ctx.enter_context(tc.tile_pool(name="big", bufs=1))

        # ---- small inputs first on the queue, then the mask stream ----
        e2f2_sb = const.tile([PJ, 2 * NJT], f32, tag="e2f2")
        nc.sync.dma_start(e2f2_sb[:], e2f2)
        e1b_sb = big.tile([PJ, N], bf16, tag="e1b")
        nc.sync.dma_start(e1b_sb[:], e1b)
        hcat_sb = const.tile([PJ, NJT * HC], bf16, tag="hcat")
        nc.sync.dma_start(hcat_sb[:], hcat)

        num_pool = ctx.enter_context(
            tc.tile_pool(name="num_psum", bufs=1, space="PSUM")
        )
        numT_ps = num_pool.tile([DOUT + 1, N], f32, tag="numt")

        def emit_matmuls(t, src, first, last):
            lhsT = hcat_sb[:, t * HC : t * HC + DOUT + 1]
            for c in range(NCH):
                sl = slice(c * FCH, (c + 1) * FCH)
                nc.tensor.matmul(
                    numT_ps[:, sl], lhsT, src[:, sl],
                    start=first, stop=last,
                )

        if ACCUM:
            # mask folded into a gpsimd (SWDGE) DMA: q = min(q, mask{0,BIG})
            qpool = ctx.enter_context(tc.tile_pool(name="q", bufs=6))
            for t in range(NJT):
                q_sb = qpool.tile([PJ, N], bf16, tag="q")
                nc.vector.tensor_scalar(
                    q_sb[:],
                    e1b_sb[:],
                    e2f2_sb[:, t : t + 1],
                    e2f2_sb[:, NJT + t : NJT + t + 1],
                    op0=ALU.mult,
                    op1=ALU.max,
                )
                acc = nc.gpsimd.dma_start(
                    q_sb[:],
                    maskt[t * PJ : (t + 1) * PJ, :],
                    accum_op=ALU.mult,
                )
                acc.ins.mode = "CCE"  # walrus rejects cce_op under Copy mode
                emit_matmuls(t, q_sb, first=(t == 0), last=(t == NJT - 1))
        else:
            mpool = ctx.enter_context(tc.tile_pool(name="mask", bufs=NJT))
            mask_sb = []
            for t in range(NJT):
                mb_t = mpool.tile([PJ, N], bf16, tag="mb")
                nc.sync.dma_start(mb_t[:], maskt[t * PJ : (t + 1) * PJ, :])
                mask_sb.append(mb_t)

            qpool = ctx.enter_context(tc.tile_pool(name="q", bufs=3))
            # gpsimd consumes its q tiles slowly -- give them a dedicated
            # pool so no later DVE tensor_scalar WAR-stalls on gpsimd
            gqpool = ctx.enter_context(
                tc.tile_pool(name="gq", bufs=len(GP_TILES))
            )
            ppool = ctx.enter_context(tc.tile_pool(name="probs", bufs=3))
            gpool = ctx.enter_context(
                tc.tile_pool(name="gp_probs", bufs=len(GP_TILES))
            )
            gp_srcs = {}
            for t in range(NJT):
                pool_t = gqpool if t in GP_TILES else qpool
                q_sb = pool_t.tile([PJ, N], bf16, tag="q")
                nc.vector.tensor_scalar(
                    q_sb[:],
                    e1b_sb[:],
                    e2f2_sb[:, t : t + 1],
                    e2f2_sb[:, NJT + t : NJT + t + 1],
                    op0=ALU.mult,
                    op1=ALU.max,
                )
                if t in GP_TILES:
                    p_sb = gpool.tile([PJ, N], bf16, tag="gp_p")
                    nc.gpsimd.tensor_tensor(
                        p_sb[:], mask_sb[t][:], q_sb[:], op=ALU.mult
                    )
                    gp_srcs[t] = p_sb
                else:
                    p_sb = ppool.tile([PJ, N], bf16, tag="p")
                    nc.vector.tensor_tensor(
                        p_sb[:], mask_sb[t][:], q_sb[:], op=ALU.mult
                    )
                    emit_matmuls(t, p_sb, first=(t == 0), last=False)
            # gpsimd tiles' matmuls go last so the in-order PE queue never
            # stalls on the slower gpsimd multiplies mid-stream
            for k, t in enumerate(GP_TILES):
                emit_matmuls(
                    t, gp_srcs[t], first=False, last=(k == len(GP_TILES) - 1)
                )

        # ---- drain numT and store; divide+transpose happen on host ----
        numt_sb = big.tile([DOUT + 1, N], f32, tag="numt_sb")
        for c in range(NCH):
            sl = slice(c * FCH, (c + 1) * FCH)
            nc.scalar.copy(numt_sb[:, sl], numT_ps[:, sl])
            nc.sync.dma_start(out[:, sl], numt_sb[:, sl])

    nc.compile()
    return nc


def _prep_inputs(x, adj, W, a):
    bf = ml_dtypes.bfloat16
    x = np.asarray(x, dtype=np.float32)
    W = np.ascontiguousarray(np.asarray(W, dtype=np.float32))
    a = np.asarray(a, dtype=np.float32)

    h = x @ W                                   # [B,N,DOUT]
    s1 = h @ a[:DOUT]                           # [B,N]
    s2 = h @ a[DOUT:]                           # [B,N]

    adjT = np.asarray(adj).T > 0                # [j,i] layout
    mask_bf = adjT.astype(bf)                   # {0,1}

    in_maps = []
    for b in range(NCORES):
        hcat = np.zeros((N, HC), dtype=bf)
        hcat[:, :DOUT] = h[b].astype(bf)
        hcat[:, DOUT] = bf(1.0)
        # pre-tiled [128, 16*66]: partition p, tile t = row t*128+p
        hcat_t = np.ascontiguousarray(
            hcat.reshape(NJT, PJ, HC).transpose(1, 0, 2).reshape(PJ, NJT * HC)
        )
        e1p = np.exp(0.8 * s1[b]).astype(bf)
        e1b = np.ascontiguousarray(np.broadcast_to(e1p[None, :], (PJ, N)))
        e2 = np.exp(s2[b]).astype(np.float32).reshape(NJT, PJ).T
        f2 = np.exp(0.2 * s2[b]).astype(np.float32).reshape(NJT, PJ).T
        e2f2 = np.ascontiguousarray(np.concatenate([e2, f2], axis=1))
        in_maps.append(
            {"maskt": mask_bf, "hcat": hcat_t, "e1b": e1b, "e2f2": e2f2}
        )
    return in_maps


def kernel(x, adj, W, a):
    global _COMPILED, LAST_RESULT
    from concourse import bass_utils

    x = np.asarray(x)
    adj = np.asarray(adj)
    assert x.shape == (B, N, DIN) and adj.shape == (N, N)

    if _COMPILED is None:
        _COMPILED = _build_nc()
    nc = _COMPILED

    in_maps = _prep_inputs(x, adj, W, a)
    res = bass_utils.run_bass_kernel_spmd(
        nc,
        in_maps,
        core_ids=list(range(NCORES)),
        trace=bool(int(os.environ.get("GAT_TRACE", "0"))),
    )
    LAST_RESULT = res
    out = np.empty((B, N, DOUT), dtype=np.float32)
    for b in range(NCORES):
        numt = res.results[b]["out"]            # [DOUT+1, N] f32
        out[b] = (numt[:DOUT] / numt[DOUT : DOUT + 1]).T
    return out
